# revision 28
# baseline (speedup 1.0000x reference)
"""Trainium2 Bass kernel: multi-head attention with relative-position bias.

Problem shapes: x [8, 1024, 768], H=12 heads, d=64.
Strategy: data-parallel over batch (1 element per NeuronCore, 8 cores).
All matmuls in bf16 (f32 PSUM accumulation). Host prep:
  - weights transposed to [C, *] feature-major; q-scale folded into Wq/q_bias
  - qkv weight columns reordered to [q0|k0|q1|k1|...|q5|k5|v] so the q0/k0
    slices (needed first) arrive in a small leading DMA
  - relative-position bias gather done as exp(table)[idx] -> bf16, streamed
    from HBM and folded into softmax multiplicatively:
    softmax(s + b) = norm(exp(s) * exp(b))   (no row-max needed: |s| < ~10)
  - eb tiles permuted to [t, jc, i-half, j, (x0 512 | x1 512)] to match the
    score-psum layout below
Attention computed transposed (sT[j, i]) so softmax sums run along the PE
contraction: the PV matmul uses stationary [v | 1], giving the denominator as
an extra psum row for free.

ALL matmuls run in the single 128x128 PE tiling mode: the score matmuls use
K=128 stationaries zero-padded per head (kz tiles: head x's k dims in rows
x*64..x*64+64, zeros elsewhere; the moving operand is the full two-head q
tile, the zero rows kill the cross-head terms). This avoids the
64x128-row-tiling <-> 128x128 mode switches (each forces a PE array drain)
that otherwise occur 4x per score group.

Phase B is a single global software pipeline over all 48 (head-pair, jc)
groups. Score psum tiles are grouped BY i-HALF: tileA holds
[s_x0(i<512) | s_x1(i<512)]; one exp (scalar engine) and one bf16 multiply
(vs the host-permuted eb tile, on the vector engine) cover both heads.
PV matmuls trail by ~3 groups via a pending queue; v-projection tiles and
the remaining q/k projection tiles are inserted as PE filler inside early
groups. Normalization uses a DRAM bounce to transpose the denominator row
into a [16,128] reciprocal; evacuation copies are split scalar/vector.
"""
import sys
import numpy as np

sys.path.insert(0, "/opt/trn_rl_repo")

import ml_dtypes

BF16 = ml_dtypes.bfloat16

B, N, C = 8, 1024, 768
H, D = 12, 64
N_CORES = 8
NT = N // 128        # 8 token tiles
CT = C // 128        # 6 feature tiles
OT = 3 * C // 128    # 18 qkv output feature tiles

_cache = {}


def _install_axon_shim():
    """The image's antenv lacks axon_hooks; register the NTFF profile hook so
    run_bass_kernel_spmd(trace=True) works. Safe no-op outside axon."""
    import types

    if "antenv.axon_hooks" not in sys.modules:
        try:
            import antenv
            from trn_agent_boot.trn_boot import _ntff_profile_via_ctypes
        except ImportError:
            return
        mod = types.ModuleType("antenv.axon_hooks")
        _hook = [None]
        mod.set_axon_ntff_profile_hook = lambda h: _hook.__setitem__(0, h)
        mod.get_axon_ntff_profile_hook = lambda: _hook[0]
        sys.modules["antenv.axon_hooks"] = mod
        antenv.axon_hooks = mod
        try:
            mod.set_axon_ntff_profile_hook(
                _ntff_profile_via_ctypes("/opt/axon/libaxon_pjrt.so")
            )
        except Exception:
            pass
    from concourse import bass_utils

    bass_utils.upload_artifacts = lambda tmpdir: tmpdir


def build_nc():
    from concourse import bacc, mybir, tile
    from concourse.tile import add_dep_helper

    f32 = mybir.dt.float32
    bf16 = mybir.dt.bfloat16
    AF = mybir.ActivationFunctionType

    nc = bacc.Bacc("TRN2", target_bir_lowering=False, debug=False,
                   num_devices=N_CORES)

    xt_d = nc.dram_tensor("xt", [C, N], bf16, kind="ExternalInput")
    # device column order: [q0|k0|q1|k1|...|q5|k5 | v(768)]
    wqkvt_d = nc.dram_tensor("wqkvt", [C, 3 * C], bf16, kind="ExternalInput")
    qkvb_d = nc.dram_tensor("qkvb", [3 * C], f32, kind="ExternalInput")
    vb_d = nc.dram_tensor("vb", [C], f32, kind="ExternalInput")
    wprojt_d = nc.dram_tensor("wprojt", [C, C], bf16, kind="ExternalInput")
    pbias_d = nc.dram_tensor("pbias", [C], f32, kind="ExternalInput")
    # eb tiles: [(t*8+jc)*2 + i-half, j, (x0 i-half | x1 i-half)]
    ebt_d = nc.dram_tensor("ebt", [96, 128, N], bf16, kind="ExternalInput")
    out_d = nc.dram_tensor("out", [N, C], bf16, kind="ExternalOutput")

    with tile.TileContext(nc) as tc:
        with (
            tc.tile_pool(name="persist", bufs=1) as persist,
            tc.tile_pool(name="work", bufs=1) as work,
            tc.tile_pool(name="dram", bufs=2, space="DRAM") as dpool,
            tc.tile_pool(name="psum", bufs=1, space="PSUM") as psum,
        ):
            # ---- resident tiles ----
            # q feature-major [128=(x,d), N] per pair
            q_sb = [persist.tile([128, N], bf16, tag=f"q{i}", name=f"q{i}")
                    for i in range(6)]
            # k stationaries, zero-padded per head: kz[2t+x] holds head x's
            # k dims in rows x*64..x*64+64, zeros elsewhere -> K=128 matmuls
            kz_sb = [persist.tile([128, N], bf16, tag=f"kz{i}", name=f"kz{i}")
                     for i in range(12)]
            # v token-major, 12 groups of (64 vals + 1 one) per token tile
            v_sb = [persist.tile([128, H * 65], bf16, tag=f"v{i}", name=f"v{i}")
                    for i in range(NT)]
            # attention output (pre-proj), feature-major
            ot_sb = [persist.tile([128, N], bf16, tag=f"ot{i}", name=f"ot{i}")
                     for i in range(CT)]
            # small constants
            qkvb_sb = persist.tile([128, OT], f32, tag="qkvb")
            vb_bc = persist.tile([128, C], f32, tag="vb_bc")
            pb_bc = persist.tile([128, C], f32, tag="pb_bc")
            ones_sb = persist.tile([128, 512], bf16, tag="ones")
            nc.vector.memset(ones_sb[:], 1.0)
            # zero halves of the kz stationaries (written once). GPSIMD:
            # keeps the vector queue free for the ramp-phase evacuations.
            for t in range(6):
                nc.gpsimd.memset(kz_sb[2 * t + 0][64:128, :], 0.0)
                nc.gpsimd.memset(kz_sb[2 * t + 1][0:64, :], 0.0)

            # ---- input DMAs, priority-ordered ----
            # first: x and the narrow q0/k0 weight slice, so compute starts asap
            xt_sb = [work.tile([128, N], bf16, tag=f"xt{i}", name=f"xt{i}")
                     for i in range(CT)]
            wqkv0_sb = [work.tile([128, 256], bf16, tag=f"wq0{i}", name=f"wq0{i}")
                        for i in range(CT)]
            wv_sb = [work.tile([128, C], bf16, tag=f"wv{i}", name=f"wv{i}")
                     for i in range(CT)]
            wqk2_sb = [work.tile([128, 2 * C - 256], bf16, tag=f"wqr{i}",
                                 name=f"wqr{i}") for i in range(CT)]
            first_dmas = []
            for ct in range(CT):
                first_dmas.append(nc.sync.dma_start(
                    xt_sb[ct][:], xt_d.ap()[ct * 128:(ct + 1) * 128, :]))
                first_dmas.append(nc.sync.dma_start(
                    wqkv0_sb[ct][:], wqkvt_d.ap()[ct * 128:(ct + 1) * 128, 0:256]))
            # qkvb needed first (q0 evac ~16us); the wide vb/pb broadcast
            # DMAs (~2us each) are deferred into the loop so they don't delay
            # the first eb tiles
            nc.sync.dma_start(qkvb_sb[:], qkvb_d.ap().rearrange("(t p) -> p t", p=128))

            # wv / wqk2 / wproj DMAs are deferred into the group loop (queue
            # order: x+wqkv0, eb(0), wv, eb(1), eb(2)+wqk2, ..., eb(12)+wproj)
            # so the eb stream starts as early as possible
            wproj_sb = [persist.tile([128, C], bf16, tag=f"wp{i}", name=f"wp{i}")
                        for i in range(CT)]

            # ================= qkv projection emitters =================
            def qk_w_ap(ct, o):
                if o < 2:
                    return wqkv0_sb[ct][:, o * 128:(o + 1) * 128]
                return wqk2_sb[ct][:, (o - 2) * 128:(o - 1) * 128]

            # q,k feature-major: qkvT[o, n] = sum_c WT[c, o] * xT[c, n]
            # emitted in two halves so an insert can straddle a score group
            def emit_qk_half(o, half, ps):
                for ct in (range(0, 3) if half == 0 else range(3, CT)):
                    for h2 in range(2):
                        nc.tensor.matmul(
                            ps[:, h2 * 512:(h2 + 1) * 512],
                            qk_w_ap(ct, o),
                            xt_sb[ct][:, h2 * 512:(h2 + 1) * 512],
                            start=(ct == 0), stop=(ct == CT - 1),
                            skip_group_check=True,
                        )
                if half == 1:
                    t = o // 2
                    if o % 2 == 0:  # q: per-partition bias add
                        nc.vector.tensor_scalar_add(q_sb[t][:], ps[:],
                                                    qkvb_sb[:, o:o + 1])
                    else:  # k: bias is identically zero -> plain copies into
                        # the data rows of the two zero-padded stationaries.
                        # (Tried on scalar: delays the exp stream there and
                        # costs more than it saves on the vector queue.)
                        nc.vector.tensor_copy(kz_sb[2 * t + 0][0:64, :],
                                              ps[0:64, :])
                        nc.vector.tensor_copy(kz_sb[2 * t + 1][64:128, :],
                                              ps[64:128, :])

            def emit_qk_tile(o):
                ps = psum.tile([128, N], f32, tag="big", name="psa", bufs=2)
                emit_qk_half(o, 0, ps)
                emit_qk_half(o, 1, ps)

            # v token-major: v[n, vd] = sum_c xT[c, n] * WT[c, 2C+vd]
            # NOTE: 384-wide matmul outputs must start at 512-aligned psum
            # offsets (a matmul output may not cross a 2KB PSUM bank).

            def emit_v_half(nt, half, ps):
                for ct in (range(0, 3) if half == 0 else range(3, CT)):
                    for g2 in range(2):
                        nc.tensor.matmul(
                            ps[:, g2 * 512:g2 * 512 + 384],
                            xt_sb[ct][:, nt * 128:(nt + 1) * 128],
                            wv_sb[ct][:, g2 * 384:(g2 + 1) * 384],
                            start=(ct == 0), stop=(ct == CT - 1),
                            skip_group_check=True,
                        )
                if half == 1:
                    emit_v_evac(nt, ps)

            def emit_v_tile(nt):
                # v tiles 0..4 use the pv psum banks (idle until the first
                # PV group pops at slot PV_LAG): keeps the "big" ring free
                # for the score tiles during the pair-0 ramp
                if nt <= 4:
                    ps = psum.tile([128, N], f32, tag=f"pv{nt % 2}",
                                   name=f"psv{nt}", bufs=1)
                else:
                    ps = psum.tile([128, N], f32, tag="big", name="psv",
                                   bufs=2)
                emit_v_half(nt, 0, ps)
                emit_v_half(nt, 1, ps)

            def emit_v_evac(nt, ps):
                v_view = v_sb[nt][:].rearrange("p (g c) -> p g c", c=65)
                ps_view = (ps[:].rearrange("p (g c) -> p g c", g=2)[:, :, 0:384]
                           .rearrange("p g (h c) -> p g h c", c=64))
                nc.vector.tensor_add(
                    v_view[:, :, 0:64].rearrange("p (g h) c -> p g h c", g=2),
                    ps_view,
                    vb_bc[:].rearrange("p (g h c) -> p g h c", g=2, c=64),
                )
                nc.vector.memset(v_view[:, :, 64:65], 1.0)

            # PE warm-up: a few junk matmuls bridging the gap until the first
            # x/weight tiles land; the qkv matmuls then keep the PE busy
            # through the HAM warm-up window themselves. More warm-up would
            # head-of-line block the real work (PE queue is strict FIFO).
            warm_ps = psum.tile([128, N], f32, tag="big", name="warm", bufs=2)
            for _ in range(4):
                nc.tensor.matmul(warm_ps[:, 0:512], ones_sb[:, 0:128],
                                 ones_sb[:, 0:512], start=True, stop=True,
                                 skip_group_check=True)

            # prologue: only q0/k0 (needs just the narrow weight slice)
            emit_qk_tile(0)
            emit_qk_tile(1)

            # ================= Phase B: attention (global pipeline) ========
            groups = [(t, jc) for t in range(6) for jc in range(NT)]  # 48
            pms = {}           # (t, jc) -> [pmA, pmB]  (A/B = i-halves)
            pv = {}            # t -> [x] psum tiles ([128,1024], rows 0:65)
            pv_pending = []    # group indices whose PV is not yet emitted
            # Emit PV for group g at slot >= g + PV_LAG. The PE queue is
            # strict FIFO: a PV matmul whose pm isn't ready head-of-line
            # blocks everything behind it. Lag 5 puts the last groups' PVs
            # behind the NEXT pair's score matmuls in the queue, so the PE
            # keeps streaming across pair boundaries while exp/mul catch up.
            # (pm pool: live range is ~6 groups x 2 tiles <= 14 bufs.)
            PV_LAG = 5
            # PE filler inserted inside groups:
            #   v tiles at (0, jc); q/k tiles for pair t+1 inside pair t
            inserts = {}
            for jc in range(NT):
                inserts.setdefault((0, jc), []).append(("v", jc))
            inserts.setdefault((0, 5), []).append(("qk", 2))
            inserts.setdefault((0, 6), []).append(("qk", 3))
            for t in range(1, 5):
                inserts.setdefault((t, 2), []).append(("qk", 2 * (t + 1)))
                # (t, 5) leaves the k evac a full group of margin before the
                # next pair's first score matmul reads the kz tiles
                inserts.setdefault((t, 5), []).append(("qk", 2 * (t + 1) + 1))

            def emit_pv_group(gi):
                t, jc = groups[gi]
                if jc == 0:
                    pv[t] = [psum.tile([128, N], f32, tag=f"pv{x}",
                                       name=f"pv{x}", bufs=1)
                             for x in range(2)]
                for x in range(2):
                    g = 2 * t + x
                    for ic in range(2):
                        nc.tensor.matmul(
                            pv[t][x][0:65, ic * 512:(ic + 1) * 512],
                            v_sb[jc][:, g * 65:(g + 1) * 65],
                            pms[(t, jc)][ic][:, x * 512:(x + 1) * 512],
                            start=(jc == 0), stop=(jc == NT - 1),
                            skip_group_check=True,
                        )
                if jc == NT - 1:
                    emit_norm(t)

            def emit_norm(t):
                # evacuate both pv tiles into one wide staging tile (cols
                # x-major); row 64 holds the denominators -> one DMA to DRAM,
                # one [16,128] reshape, one reciprocal, one wide stride-0
                # broadcast back, two scaling multiplies.
                u2 = work.tile([65, 2048], bf16, tag="ustage", name="ustage",
                               bufs=2)
                nc.scalar.copy(u2[0:65, 0:1024], pv[t][0][0:65, :])
                nc.vector.tensor_copy(u2[0:65, 1024:2048], pv[t][1][0:65, :])
                # denominator row -> DRAM bounce -> [16,128] compact ->
                # reciprocal -> DRAM bounce -> broadcast across 64 partitions
                # (partition-redistributing / stride-0 APs require DRAM)
                denom_d = dpool.tile([1, 2048], bf16, tag="denom_d",
                                     name="denom_d")
                nc.sync.dma_start(denom_d[0:1, :], u2[64:65, :])
                dstage = work.tile([16, 128], bf16, tag="dstage",
                                   name="dstage", bufs=2)
                nc.sync.dma_start(
                    dstage[:],
                    denom_d[:].rearrange("a b -> (a b)").rearrange(
                        "(p c) -> p c", p=16))
                rstage = work.tile([16, 128], bf16, tag="rstage",
                                   name="rstage", bufs=2)
                with nc.allow_low_precision("softmax denom recip, 2e-2 gate"):
                    nc.vector.reciprocal(rstage[:], dstage[:])
                rd = dpool.tile([16, 128], bf16, tag="rd", name="rd")
                nc.sync.dma_start(rd[:], rstage[:])
                rb2 = work.tile([64, 2048], bf16, tag="rb", name="rb", bufs=2)
                nc.sync.dma_start(
                    rb2[:],
                    rd[:].rearrange("p c -> (p c)").unsqueeze(0)
                    .to_broadcast([64, 2048]))
                for x in range(2):
                    nc.vector.tensor_mul(
                        ot_sb[t][x * 64:(x + 1) * 64, :],
                        u2[0:64, x * 1024:(x + 1) * 1024],
                        rb2[:, x * 1024:(x + 1) * 1024],
                    )

            for gi, (t, jc) in enumerate(groups):
                # eb bias tiles for this group (one per i-half, both heads)
                eb = [work.tile([128, N], bf16, tag="eb", name="eb", bufs=12)
                      for _ in range(2)]
                for a in range(2):
                    nc.sync.dma_start(
                        eb[a][:], ebt_d.ap()[(t * NT + jc) * 2 + a, :, :])
                if gi == 0:
                    for ct in range(CT):
                        nc.sync.dma_start(
                            wv_sb[ct][:],
                            wqkvt_d.ap()[ct * 128:(ct + 1) * 128, 2 * C:])
                    nc.sync.dma_start(
                        vb_bc[:], vb_d.ap().unsqueeze(0).to_broadcast([128, C]))
                if gi == 2:
                    # remaining q/k weights: first consumer is the ("qk", 2)
                    # insert at group 5; enqueue behind the first few eb tiles
                    for ct in range(CT):
                        nc.sync.dma_start(
                            wqk2_sb[ct][:],
                            wqkvt_d.ap()[ct * 128:(ct + 1) * 128, 256:2 * C])
                if gi == 12:
                    # proj weights + bias: needed only in phase C; enqueue
                    # behind the first dozen eb tiles
                    for ct in range(CT):
                        nc.sync.dma_start(
                            wproj_sb[ct][:],
                            wprojt_d.ap()[ct * 128:(ct + 1) * 128, :])
                    nc.sync.dma_start(
                        pb_bc[:],
                        pbias_d.ap().unsqueeze(0).to_broadcast([128, C]))
                # scores grouped by i-half: tile a holds both heads' scores
                # for i in [a*512, (a+1)*512). K=128 matmuls (zero-padded
                # stationary) keep the PE in plain 128x128 mode; PE filler
                # (v / later q,k projection tiles) is emitted between the two
                # score tiles so it streams while exp(A) runs.
                qs = []
                for a in range(2):
                    q = psum.tile([128, N], f32, tag="big", name=f"qs{a}",
                                  bufs=2)
                    qs.append(q)
                    for x in range(2):
                        nc.tensor.matmul(
                            q[:, x * 512:(x + 1) * 512],
                            kz_sb[2 * t + x][:, jc * 128:(jc + 1) * 128],
                            q_sb[t][:, a * 512:(a + 1) * 512],
                            start=True, stop=True,
                            skip_group_check=True,
                        )
                    if a == 0:
                        for kind, idx in inserts.get((t, jc), []):
                            if kind == "v":
                                emit_v_tile(idx)
                            else:
                                emit_qk_tile(idx)
                pml = []
                for a in range(2):
                    pe = work.tile([128, N], bf16, tag="pe", name="pe", bufs=6)
                    nc.scalar.activation(pe[:], qs[a][:], AF.Exp)
                    pm = work.tile([128, N], bf16, tag="pm", name="pm", bufs=14)
                    # NOTE: GPSIMD offload of these multiplies was tried and
                    # is a net loss — GPSIMD shares the SBUF port with the
                    # DVE, and concurrent DVE tensor_tensor ops slowed 3.5x.
                    nc.vector.tensor_mul(pm[:], pe[:], eb[a][:])
                    pml.append(pm)
                pms[(t, jc)] = pml
                pv_pending.append(gi)
                # lagged PV emission (<=2 groups per slot keeps PE smooth)
                emitted = 0
                while pv_pending and pv_pending[0] <= gi - PV_LAG and emitted < 2:
                    emit_pv_group(pv_pending.pop(0))
                    emitted += 1
            # ================= Phase C: output projection =================
            # Interleave the final PV drain with ct=0..4 accumulation for the
            # first token tiles; nt=2/3 reuse the pv psum banks freed by the
            # last pair's norm copies. ct=5 (gated on ot_sb[5]) finishes each
            # held tile afterwards; nt=4..7 then run all six ct in one pass.
            def emit_proj_mms(nt, ps, cts, first, last):
                for ct in cts:
                    for oc in range(2):
                        nc.tensor.matmul(
                            ps[:, oc * 512:oc * 512 + 384],
                            ot_sb[ct][:, nt * 128:(nt + 1) * 128],
                            wproj_sb[ct][:, oc * 384:(oc + 1) * 384],
                            start=(ct == first), stop=(ct == last),
                            skip_group_check=True,
                        )

            def emit_proj_out(nt, ps):
                osb = work.tile([128, C], bf16, tag="osb", name="osb", bufs=3)
                ps_view = ps[:].rearrange("p (g c) -> p g c", g=2)[:, :, 0:384]
                nc.vector.tensor_add(
                    osb[:].rearrange("p (g c) -> p g c", g=2), ps_view,
                    pb_bc[:].rearrange("p (g c) -> p g c", g=2))
                nc.sync.dma_start(out_d.ap()[nt * 128:(nt + 1) * 128, :], osb[:])

            def proj_ps(nt):
                tag = ["big", "big", "pv0", "pv1"][nt % 4]
                return psum.tile([128, N], f32, tag=tag, name=f"psc{nt}",
                                 bufs=(2 if tag == "big" else 1))

            held4 = []
            while pv_pending:
                emit_pv_group(pv_pending.pop(0))
                if len(held4) < 2:  # nt 0/1 on the "big" ring during drain
                    nt = len(held4)
                    ps = proj_ps(nt)
                    emit_proj_mms(nt, ps, range(5), 0, CT - 1)
                    held4.append((nt, ps))
            for nt in (2, 3):  # pv banks free once norm(5)'s copies ran
                ps = proj_ps(nt)
                emit_proj_mms(nt, ps, range(5), 0, CT - 1)
                held4.append((nt, ps))
            for nt, ps in held4:
                emit_proj_mms(nt, ps, [5], 0, CT - 1)
                emit_proj_out(nt, ps)
            for nt in range(4, NT):
                ps = proj_ps(nt)
                emit_proj_mms(nt, ps, range(CT), 0, CT - 1)
                emit_proj_out(nt, ps)

    nc.compile()
    return nc


def _get_nc():
    if "nc" not in _cache:
        _install_axon_shim()
        _cache["nc"] = build_nc()
    return _cache["nc"]


def prep_inputs(x, relative_position_index, qkv_weight, q_bias, v_bias,
                proj_weight, proj_bias, rel_pos_bias_table):
    """Host-side layout prep shared by all cores + per-core shards."""
    x = np.asarray(x, np.float32)
    idx = np.asarray(relative_position_index)
    qkv_weight = np.asarray(qkv_weight, np.float32)
    q_bias = np.asarray(q_bias, np.float32)
    v_bias = np.asarray(v_bias, np.float32)
    proj_weight = np.asarray(proj_weight, np.float32)
    proj_bias = np.asarray(proj_bias, np.float32)
    tbl = np.asarray(rel_pos_bias_table, np.float32)

    scale = (C // H) ** (-0.5)
    wq = qkv_weight.copy()
    wq[:C, :] *= scale  # fold softmax scale into q projection
    wqkvt = np.ascontiguousarray(wq.T)  # [C, 3C] cols: q(768) k(768) v(768)
    # device column order: [q0|k0|q1|k1|...|q5|k5|v]
    cols = []
    for t in range(6):
        cols.append(wqkvt[:, t * 128:(t + 1) * 128])
        cols.append(wqkvt[:, C + t * 128:C + (t + 1) * 128])
    cols.append(wqkvt[:, 2 * C:])
    wqkvt_dev = np.ascontiguousarray(np.concatenate(cols, axis=1)).astype(BF16)

    qb_s = q_bias * scale
    qkvb_parts = []
    for t in range(6):
        qkvb_parts.append(qb_s[t * 128:(t + 1) * 128])
        qkvb_parts.append(np.zeros(128, np.float32))
    qkvb_parts.append(v_bias)
    qkvb = np.concatenate(qkvb_parts).astype(np.float32)

    wprojt = np.ascontiguousarray(proj_weight.T).astype(BF16)  # [C, C]

    # exp(bias) gather: ebt[h, j, i] = exp(table[idx[i, j], h]), then permute
    # to [t, jc, i-half, j, (x0 i-half | x1 i-half)]
    eb = np.exp(tbl)[idx]                                    # [i, j, H] f32
    ebt = eb.transpose(2, 1, 0)                              # [H, Nj, Ni]
    e6 = ebt.reshape(6, 2, NT, 128, 2, 512)                  # [t,x,jc,j,a,i]
    ebt_dev = np.ascontiguousarray(
        e6.transpose(0, 2, 4, 3, 1, 5)).reshape(96, 128, N).astype(BF16)

    shared = {
        "wqkvt": wqkvt_dev,
        "qkvb": qkvb,
        "vb": v_bias.astype(np.float32),
        "wprojt": wprojt,
        "pbias": proj_bias.astype(np.float32),
        "ebt": ebt_dev,
    }
    in_maps = []
    for b in range(B):
        m = dict(shared)
        m["xt"] = np.ascontiguousarray(x[b].T).astype(BF16)  # [C, N]
        in_maps.append(m)
    return in_maps


def kernel(**inputs):
    from concourse.bass_utils import run_bass_kernel_spmd

    nc = _get_nc()
    in_maps = prep_inputs(**inputs)
    res = run_bass_kernel_spmd(nc, in_maps, list(range(N_CORES)),
                               trace=False)
    _cache["last_result"] = res
    out = np.stack([res.results[b]["out"] for b in range(B)], axis=0)
    return out.astype(np.float32)


def kernel_profiled(**inputs):
    """Same as kernel() but with NTFF tracing; returns (out, BassKernelResults)."""
    from concourse.bass_utils import run_bass_kernel_spmd

    nc = _get_nc()
    in_maps = prep_inputs(**inputs)
    res = run_bass_kernel_spmd(nc, in_maps, list(range(N_CORES)), trace=True)
    out = np.stack([res.results[b]["out"] for b in range(B)], axis=0)
    return out.astype(np.float32), res


# revision 29
# speedup vs baseline: 1.1918x; 1.1918x over previous
"""Trainium2 Bass kernel: multi-head attention with relative-position bias.

Problem shapes: x [8, 1024, 768], H=12 heads, d=64.
Strategy: data-parallel over batch (1 element per NeuronCore, 8 cores).
All matmuls in bf16 (f32 PSUM accumulation). Host prep:
  - weights transposed to [C, *] feature-major; q-scale folded into Wq/q_bias
  - qkv weight columns reordered to [q0|k0|q1|k1|...|q5|k5|v] so the q0/k0
    slices (needed first) arrive in a small leading DMA
  - relative-position bias gather done as exp(table)[idx] -> bf16, streamed
    from HBM and folded into softmax multiplicatively:
    softmax(s + b) = norm(exp(s) * exp(b))   (no row-max needed: |s| < ~10)
  - eb tiles permuted to [t, jc, i-half, j, (x0 512 | x1 512)] to match the
    score-psum layout below
Attention computed transposed (sT[j, i]) so softmax sums run along the PE
contraction: the PV matmul uses stationary [v | 1], giving the denominator as
an extra psum row for free.

ALL matmuls run in the single 128x128 PE tiling mode: the score matmuls use
K=128 stationaries zero-padded per head (kz tiles: head x's k dims in rows
x*64..x*64+64, zeros elsewhere; the moving operand is the full two-head q
tile, the zero rows kill the cross-head terms). This avoids the
64x128-row-tiling <-> 128x128 mode switches (each forces a PE array drain)
that otherwise occur 4x per score group.

Phase B is a single global software pipeline over all 48 (head-pair, jc)
groups. Score psum tiles are grouped BY i-HALF: tileA holds
[s_x0(i<512) | s_x1(i<512)]; one exp (scalar engine) and one bf16 multiply
(vs the host-permuted eb tile, on the vector engine) cover both heads.
PV matmuls trail by ~3 groups via a pending queue; v-projection tiles and
the remaining q/k projection tiles are inserted as PE filler inside early
groups. Normalization uses a DRAM bounce to transpose the denominator row
into a [16,128] reciprocal; evacuation copies are split scalar/vector.
"""
import sys
import numpy as np

sys.path.insert(0, "/opt/trn_rl_repo")

import ml_dtypes

BF16 = ml_dtypes.bfloat16

B, N, C = 8, 1024, 768
H, D = 12, 64
N_CORES = 8
NT = N // 128        # 8 token tiles
CT = C // 128        # 6 feature tiles
OT = 3 * C // 128    # 18 qkv output feature tiles

_cache = {}


def _install_axon_shim():
    """The image's antenv lacks axon_hooks; register the NTFF profile hook so
    run_bass_kernel_spmd(trace=True) works. Safe no-op outside axon."""
    import types

    if "antenv.axon_hooks" not in sys.modules:
        try:
            import antenv
            from trn_agent_boot.trn_boot import _ntff_profile_via_ctypes
        except ImportError:
            return
        mod = types.ModuleType("antenv.axon_hooks")
        _hook = [None]
        mod.set_axon_ntff_profile_hook = lambda h: _hook.__setitem__(0, h)
        mod.get_axon_ntff_profile_hook = lambda: _hook[0]
        sys.modules["antenv.axon_hooks"] = mod
        antenv.axon_hooks = mod
        try:
            mod.set_axon_ntff_profile_hook(
                _ntff_profile_via_ctypes("/opt/axon/libaxon_pjrt.so")
            )
        except Exception:
            pass
    from concourse import bass_utils

    bass_utils.upload_artifacts = lambda tmpdir: tmpdir


def build_nc():
    from concourse import bacc, mybir, tile
    from concourse.tile import add_dep_helper

    f32 = mybir.dt.float32
    bf16 = mybir.dt.bfloat16
    AF = mybir.ActivationFunctionType

    nc = bacc.Bacc("TRN2", target_bir_lowering=False, debug=False,
                   num_devices=N_CORES)

    xt_d = nc.dram_tensor("xt", [C, N], bf16, kind="ExternalInput")
    # device column order: [q0|k0|q1|k1|...|q5|k5 | v(768)]
    wqkvt_d = nc.dram_tensor("wqkvt", [C, 3 * C], bf16, kind="ExternalInput")
    qkvb_d = nc.dram_tensor("qkvb", [3 * C], f32, kind="ExternalInput")
    vb_d = nc.dram_tensor("vb", [C], f32, kind="ExternalInput")
    wprojt_d = nc.dram_tensor("wprojt", [C, C], bf16, kind="ExternalInput")
    pbias_d = nc.dram_tensor("pbias", [C], f32, kind="ExternalInput")
    # eb tiles: [(t*8+jc)*2 + i-half, j, (x0 i-half | x1 i-half)]
    ebt_d = nc.dram_tensor("ebt", [96, 128, N], bf16, kind="ExternalInput")
    out_d = nc.dram_tensor("out", [N, C], bf16, kind="ExternalOutput")

    with tile.TileContext(nc) as tc:
        with (
            tc.tile_pool(name="persist", bufs=1) as persist,
            tc.tile_pool(name="work", bufs=1) as work,
            tc.tile_pool(name="dram", bufs=2, space="DRAM") as dpool,
            tc.tile_pool(name="psum", bufs=1, space="PSUM") as psum,
        ):
            # ---- resident tiles ----
            # q feature-major [128=(x,d), N] per pair
            q_sb = [persist.tile([128, N], bf16, tag=f"q{i}", name=f"q{i}")
                    for i in range(6)]
            # k stationaries, zero-padded per head: kz[2t+x] holds head x's
            # k dims in rows x*64..x*64+64, zeros elsewhere -> K=128 matmuls
            kz_sb = [persist.tile([128, N], bf16, tag=f"kz{i}", name=f"kz{i}")
                     for i in range(12)]
            # v token-major, 12 groups of (64 vals + 1 one) per token tile
            v_sb = [persist.tile([128, H * 65], bf16, tag=f"v{i}", name=f"v{i}")
                    for i in range(NT)]
            # attention output (pre-proj), feature-major
            ot_sb = [persist.tile([128, N], bf16, tag=f"ot{i}", name=f"ot{i}")
                     for i in range(CT)]
            # small constants
            qkvb_sb = persist.tile([128, OT], f32, tag="qkvb")
            vb_bc = persist.tile([128, C], f32, tag="vb_bc")
            pb_bc = persist.tile([128, C], f32, tag="pb_bc")
            ones_sb = persist.tile([128, 512], bf16, tag="ones")
            nc.vector.memset(ones_sb[:], 1.0)
            # zero halves of the kz stationaries (written once). GPSIMD:
            # keeps the vector queue free for the ramp-phase evacuations.
            for t in range(6):
                nc.gpsimd.memset(kz_sb[2 * t + 0][64:128, :], 0.0)
                nc.gpsimd.memset(kz_sb[2 * t + 1][0:64, :], 0.0)

            # ---- input DMAs, priority-ordered ----
            # first: x and the narrow q0/k0 weight slice, so compute starts asap
            xt_sb = [work.tile([128, N], bf16, tag=f"xt{i}", name=f"xt{i}")
                     for i in range(CT)]
            wqkv0_sb = [work.tile([128, 256], bf16, tag=f"wq0{i}", name=f"wq0{i}")
                        for i in range(CT)]
            wv_sb = [work.tile([128, C], bf16, tag=f"wv{i}", name=f"wv{i}")
                     for i in range(CT)]
            wqk2_sb = [work.tile([128, 2 * C - 256], bf16, tag=f"wqr{i}",
                                 name=f"wqr{i}") for i in range(CT)]
            first_dmas = []
            for ct in range(CT):
                first_dmas.append(nc.sync.dma_start(
                    xt_sb[ct][:], xt_d.ap()[ct * 128:(ct + 1) * 128, :]))
                first_dmas.append(nc.sync.dma_start(
                    wqkv0_sb[ct][:], wqkvt_d.ap()[ct * 128:(ct + 1) * 128, 0:256]))
            # qkvb needed first (q0 evac ~16us); the wide vb/pb broadcast
            # DMAs (~2us each) are deferred into the loop so they don't delay
            # the first eb tiles
            nc.sync.dma_start(qkvb_sb[:], qkvb_d.ap().rearrange("(t p) -> p t", p=128))

            # wv / wqk2 / wproj DMAs are deferred into the group loop (queue
            # order: x+wqkv0, eb(0), wv, eb(1), eb(2)+wqk2, ..., eb(12)+wproj)
            # so the eb stream starts as early as possible
            wproj_sb = [persist.tile([128, C], bf16, tag=f"wp{i}", name=f"wp{i}")
                        for i in range(CT)]

            # ================= qkv projection emitters =================
            def qk_w_ap(ct, o):
                if o < 2:
                    return wqkv0_sb[ct][:, o * 128:(o + 1) * 128]
                return wqk2_sb[ct][:, (o - 2) * 128:(o - 1) * 128]

            # q,k feature-major: qkvT[o, n] = sum_c WT[c, o] * xT[c, n]
            # emitted in two halves so an insert can straddle a score group
            def emit_qk_half(o, half, ps):
                for ct in (range(0, 3) if half == 0 else range(3, CT)):
                    for h2 in range(2):
                        nc.tensor.matmul(
                            ps[:, h2 * 512:(h2 + 1) * 512],
                            qk_w_ap(ct, o),
                            xt_sb[ct][:, h2 * 512:(h2 + 1) * 512],
                            start=(ct == 0), stop=(ct == CT - 1),
                            skip_group_check=True,
                        )
                if half == 1:
                    t = o // 2
                    if o % 2 == 0:  # q: per-partition bias add
                        nc.vector.tensor_scalar_add(q_sb[t][:], ps[:],
                                                    qkvb_sb[:, o:o + 1])
                    else:  # k: bias is identically zero -> plain copies into
                        # the data rows of the two zero-padded stationaries.
                        # (Tried on scalar: delays the exp stream there and
                        # costs more than it saves on the vector queue.)
                        nc.vector.tensor_copy(kz_sb[2 * t + 0][0:64, :],
                                              ps[0:64, :])
                        nc.vector.tensor_copy(kz_sb[2 * t + 1][64:128, :],
                                              ps[64:128, :])

            def emit_qk_tile(o):
                ps = psum.tile([128, N], f32, tag="big", name="psa", bufs=2)
                emit_qk_half(o, 0, ps)
                emit_qk_half(o, 1, ps)

            # v token-major: v[n, vd] = sum_c xT[c, n] * WT[c, 2C+vd]
            # NOTE: 384-wide matmul outputs must start at 512-aligned psum
            # offsets (a matmul output may not cross a 2KB PSUM bank).

            def emit_v_half(nt, half, ps):
                for ct in (range(0, 3) if half == 0 else range(3, CT)):
                    for g2 in range(2):
                        nc.tensor.matmul(
                            ps[:, g2 * 512:g2 * 512 + 384],
                            xt_sb[ct][:, nt * 128:(nt + 1) * 128],
                            wv_sb[ct][:, g2 * 384:(g2 + 1) * 384],
                            start=(ct == 0), stop=(ct == CT - 1),
                            skip_group_check=True,
                        )
                if half == 1:
                    emit_v_evac(nt, ps)

            def emit_v_tile(nt):
                # v tiles 0..4 use the pv psum banks (idle until the first
                # PV group pops at slot PV_LAG): keeps the "big" ring free
                # for the score tiles during the pair-0 ramp
                if nt <= 4:
                    ps = psum.tile([128, N], f32, tag=f"pv{nt % 2}",
                                   name=f"psv{nt}", bufs=1)
                else:
                    ps = psum.tile([128, N], f32, tag="big", name="psv",
                                   bufs=2)
                emit_v_half(nt, 0, ps)
                emit_v_half(nt, 1, ps)

            def emit_v_evac(nt, ps):
                v_view = v_sb[nt][:].rearrange("p (g c) -> p g c", c=65)
                ps_view = (ps[:].rearrange("p (g c) -> p g c", g=2)[:, :, 0:384]
                           .rearrange("p g (h c) -> p g h c", c=64))
                nc.vector.tensor_add(
                    v_view[:, :, 0:64].rearrange("p (g h) c -> p g h c", g=2),
                    ps_view,
                    vb_bc[:].rearrange("p (g h c) -> p g h c", g=2, c=64),
                )
                nc.vector.memset(v_view[:, :, 64:65], 1.0)

            # PE warm-up: a few junk matmuls bridging the gap until the first
            # x/weight tiles land; the qkv matmuls then keep the PE busy
            # through the HAM warm-up window themselves. More warm-up would
            # head-of-line block the real work (PE queue is strict FIFO).
            warm_ps = psum.tile([128, N], f32, tag="big", name="warm", bufs=2)
            for _ in range(4):
                nc.tensor.matmul(warm_ps[:, 0:512], ones_sb[:, 0:128],
                                 ones_sb[:, 0:512], start=True, stop=True,
                                 skip_group_check=True)

            # prologue: only q0/k0 (needs just the narrow weight slice)
            emit_qk_tile(0)
            emit_qk_tile(1)

            # ================= Phase B: attention (global pipeline) ========
            groups = [(t, jc) for t in range(6) for jc in range(NT)]  # 48
            pms = {}           # (t, jc) -> [pmA, pmB]  (A/B = i-halves)
            pv = {}            # t -> [x] psum tiles ([128,1024], rows 0:65)
            pv_pending = []    # group indices whose PV is not yet emitted
            # Emit PV for group g at slot >= g + PV_LAG. The PE queue is
            # strict FIFO: a PV matmul whose pm isn't ready head-of-line
            # blocks everything behind it. Lag 5 puts the last groups' PVs
            # behind the NEXT pair's score matmuls in the queue, so the PE
            # keeps streaming across pair boundaries while exp/mul catch up.
            # (pm pool: live range is ~6 groups x 2 tiles <= 14 bufs.)
            PV_LAG = 5
            # PE filler inserted inside groups:
            #   v tiles at (0, jc); q/k tiles for pair t+1 inside pair t
            inserts = {}
            for jc in range(NT):
                inserts.setdefault((0, jc), []).append(("v", jc))
            inserts.setdefault((0, 5), []).append(("qk", 2))
            inserts.setdefault((0, 6), []).append(("qk", 3))
            for t in range(1, 5):
                inserts.setdefault((t, 2), []).append(("qk", 2 * (t + 1)))
                # (t, 5) leaves the k evac a full group of margin before the
                # next pair's first score matmul reads the kz tiles
                inserts.setdefault((t, 5), []).append(("qk", 2 * (t + 1) + 1))

            def emit_pv_group(gi):
                t, jc = groups[gi]
                if jc == 0:
                    pv[t] = [psum.tile([128, N], f32, tag=f"pv{x}",
                                       name=f"pv{x}", bufs=1)
                             for x in range(2)]
                for x in range(2):
                    g = 2 * t + x
                    for ic in range(2):
                        nc.tensor.matmul(
                            pv[t][x][0:65, ic * 512:(ic + 1) * 512],
                            v_sb[jc][:, g * 65:(g + 1) * 65],
                            pms[(t, jc)][ic][:, x * 512:(x + 1) * 512],
                            start=(jc == 0), stop=(jc == NT - 1),
                            skip_group_check=True,
                        )
                if jc == NT - 1:
                    emit_norm(t)

            def emit_norm(t):
                # evacuate both pv tiles into one wide staging tile (cols
                # x-major); row 64 holds the denominators -> one DMA to DRAM,
                # one [16,128] reshape, one reciprocal, one wide stride-0
                # broadcast back, two scaling multiplies.
                u2 = work.tile([65, 2048], bf16, tag="ustage", name="ustage",
                               bufs=2)
                nc.scalar.copy(u2[0:65, 0:1024], pv[t][0][0:65, :])
                nc.vector.tensor_copy(u2[0:65, 1024:2048], pv[t][1][0:65, :])
                # denominator row -> DRAM bounce -> [16,128] compact ->
                # reciprocal -> DRAM bounce -> broadcast across 64 partitions
                # (partition-redistributing / stride-0 APs require DRAM)
                denom_d = dpool.tile([1, 2048], bf16, tag="denom_d",
                                     name="denom_d")
                nc.sync.dma_start(denom_d[0:1, :], u2[64:65, :])
                dstage = work.tile([16, 128], bf16, tag="dstage",
                                   name="dstage", bufs=2)
                nc.sync.dma_start(
                    dstage[:],
                    denom_d[:].rearrange("a b -> (a b)").rearrange(
                        "(p c) -> p c", p=16))
                rstage = work.tile([16, 128], bf16, tag="rstage",
                                   name="rstage", bufs=2)
                with nc.allow_low_precision("softmax denom recip, 2e-2 gate"):
                    nc.vector.reciprocal(rstage[:], dstage[:])
                rd = dpool.tile([16, 128], bf16, tag="rd", name="rd")
                nc.sync.dma_start(rd[:], rstage[:])
                rb2 = work.tile([64, 2048], bf16, tag="rb", name="rb", bufs=2)
                nc.sync.dma_start(
                    rb2[:],
                    rd[:].rearrange("p c -> (p c)").unsqueeze(0)
                    .to_broadcast([64, 2048]))
                for x in range(2):
                    nc.vector.tensor_mul(
                        ot_sb[t][x * 64:(x + 1) * 64, :],
                        u2[0:64, x * 1024:(x + 1) * 1024],
                        rb2[:, x * 1024:(x + 1) * 1024],
                    )

            for gi, (t, jc) in enumerate(groups):
                # eb bias tiles for this group (one per i-half, both heads)
                eb = [work.tile([128, N], bf16, tag="eb", name="eb", bufs=12)
                      for _ in range(2)]
                if gi == 0:
                    # wv before eb(0): the v0 insert consumes wv at ~11us,
                    # while the first eb tile isn't read until ~16us
                    for ct in range(CT):
                        nc.sync.dma_start(
                            wv_sb[ct][:],
                            wqkvt_d.ap()[ct * 128:(ct + 1) * 128, 2 * C:])
                    nc.sync.dma_start(
                        vb_bc[:], vb_d.ap().unsqueeze(0).to_broadcast([128, C]))
                for a in range(2):
                    nc.sync.dma_start(
                        eb[a][:], ebt_d.ap()[(t * NT + jc) * 2 + a, :, :])
                if gi == 2:
                    # remaining q/k weights: first consumer is the ("qk", 2)
                    # insert at group 5; enqueue behind the first few eb tiles
                    for ct in range(CT):
                        nc.sync.dma_start(
                            wqk2_sb[ct][:],
                            wqkvt_d.ap()[ct * 128:(ct + 1) * 128, 256:2 * C])
                if gi == 12:
                    # proj weights + bias: needed only in phase C; enqueue
                    # behind the first dozen eb tiles
                    for ct in range(CT):
                        nc.sync.dma_start(
                            wproj_sb[ct][:],
                            wprojt_d.ap()[ct * 128:(ct + 1) * 128, :])
                    nc.sync.dma_start(
                        pb_bc[:],
                        pbias_d.ap().unsqueeze(0).to_broadcast([128, C]))
                # scores grouped by i-half: tile a holds both heads' scores
                # for i in [a*512, (a+1)*512). K=128 matmuls (zero-padded
                # stationary) keep the PE in plain 128x128 mode; PE filler
                # (v / later q,k projection tiles) is emitted between the two
                # score tiles so it streams while exp(A) runs.
                qs = []
                for a in range(2):
                    q = psum.tile([128, N], f32, tag="big", name=f"qs{a}",
                                  bufs=2)
                    qs.append(q)
                    for x in range(2):
                        nc.tensor.matmul(
                            q[:, x * 512:(x + 1) * 512],
                            kz_sb[2 * t + x][:, jc * 128:(jc + 1) * 128],
                            q_sb[t][:, a * 512:(a + 1) * 512],
                            start=True, stop=True,
                            skip_group_check=True,
                        )
                    if a == 0:
                        for kind, idx in inserts.get((t, jc), []):
                            if kind == "v":
                                emit_v_tile(idx)
                            else:
                                emit_qk_tile(idx)
                pml = []
                for a in range(2):
                    pe = work.tile([128, N], bf16, tag="pe", name="pe", bufs=6)
                    nc.scalar.activation(pe[:], qs[a][:], AF.Exp)
                    pm = work.tile([128, N], bf16, tag="pm", name="pm", bufs=14)
                    # NOTE: GPSIMD offload of these multiplies was tried and
                    # is a net loss — GPSIMD shares the SBUF port with the
                    # DVE, and concurrent DVE tensor_tensor ops slowed 3.5x.
                    nc.vector.tensor_mul(pm[:], pe[:], eb[a][:])
                    pml.append(pm)
                pms[(t, jc)] = pml
                pv_pending.append(gi)
                # lagged PV emission (<=2 groups per slot keeps PE smooth)
                emitted = 0
                while pv_pending and pv_pending[0] <= gi - PV_LAG and emitted < 2:
                    emit_pv_group(pv_pending.pop(0))
                    emitted += 1
            # ================= Phase C: output projection =================
            # Interleave the final PV drain with ct=0..4 accumulation for the
            # first token tiles; nt=2/3 reuse the pv psum banks freed by the
            # last pair's norm copies. ct=5 (gated on ot_sb[5]) finishes each
            # held tile afterwards; nt=4..7 then run all six ct in one pass.
            def emit_proj_mms(nt, ps, cts, first, last):
                for ct in cts:
                    for oc in range(2):
                        nc.tensor.matmul(
                            ps[:, oc * 512:oc * 512 + 384],
                            ot_sb[ct][:, nt * 128:(nt + 1) * 128],
                            wproj_sb[ct][:, oc * 384:(oc + 1) * 384],
                            start=(ct == first), stop=(ct == last),
                            skip_group_check=True,
                        )

            def emit_proj_out(nt, ps):
                osb = work.tile([128, C], bf16, tag="osb", name="osb", bufs=3)
                ps_view = ps[:].rearrange("p (g c) -> p g c", g=2)[:, :, 0:384]
                nc.vector.tensor_add(
                    osb[:].rearrange("p (g c) -> p g c", g=2), ps_view,
                    pb_bc[:].rearrange("p (g c) -> p g c", g=2))
                nc.sync.dma_start(out_d.ap()[nt * 128:(nt + 1) * 128, :], osb[:])

            def proj_ps(nt):
                tag = ["big", "big", "pv0", "pv1"][nt % 4]
                return psum.tile([128, N], f32, tag=tag, name=f"psc{nt}",
                                 bufs=(2 if tag == "big" else 1))

            held4 = []
            while pv_pending:
                emit_pv_group(pv_pending.pop(0))
                if len(held4) < 2:  # nt 0/1 on the "big" ring during drain
                    nt = len(held4)
                    ps = proj_ps(nt)
                    emit_proj_mms(nt, ps, range(5), 0, CT - 1)
                    held4.append((nt, ps))
            for nt in (2, 3):  # pv banks free once norm(5)'s copies ran
                ps = proj_ps(nt)
                emit_proj_mms(nt, ps, range(5), 0, CT - 1)
                held4.append((nt, ps))
            for nt, ps in held4:
                emit_proj_mms(nt, ps, [5], 0, CT - 1)
                emit_proj_out(nt, ps)
            for nt in range(4, NT):
                ps = proj_ps(nt)
                emit_proj_mms(nt, ps, range(CT), 0, CT - 1)
                emit_proj_out(nt, ps)

    nc.compile()
    return nc


def _get_nc():
    if "nc" not in _cache:
        _install_axon_shim()
        _cache["nc"] = build_nc()
    return _cache["nc"]


def prep_inputs(x, relative_position_index, qkv_weight, q_bias, v_bias,
                proj_weight, proj_bias, rel_pos_bias_table):
    """Host-side layout prep shared by all cores + per-core shards."""
    x = np.asarray(x, np.float32)
    idx = np.asarray(relative_position_index)
    qkv_weight = np.asarray(qkv_weight, np.float32)
    q_bias = np.asarray(q_bias, np.float32)
    v_bias = np.asarray(v_bias, np.float32)
    proj_weight = np.asarray(proj_weight, np.float32)
    proj_bias = np.asarray(proj_bias, np.float32)
    tbl = np.asarray(rel_pos_bias_table, np.float32)

    scale = (C // H) ** (-0.5)
    wq = qkv_weight.copy()
    wq[:C, :] *= scale  # fold softmax scale into q projection
    wqkvt = np.ascontiguousarray(wq.T)  # [C, 3C] cols: q(768) k(768) v(768)
    # device column order: [q0|k0|q1|k1|...|q5|k5|v]
    cols = []
    for t in range(6):
        cols.append(wqkvt[:, t * 128:(t + 1) * 128])
        cols.append(wqkvt[:, C + t * 128:C + (t + 1) * 128])
    cols.append(wqkvt[:, 2 * C:])
    wqkvt_dev = np.ascontiguousarray(np.concatenate(cols, axis=1)).astype(BF16)

    qb_s = q_bias * scale
    qkvb_parts = []
    for t in range(6):
        qkvb_parts.append(qb_s[t * 128:(t + 1) * 128])
        qkvb_parts.append(np.zeros(128, np.float32))
    qkvb_parts.append(v_bias)
    qkvb = np.concatenate(qkvb_parts).astype(np.float32)

    wprojt = np.ascontiguousarray(proj_weight.T).astype(BF16)  # [C, C]

    # exp(bias) gather: ebt[h, j, i] = exp(table[idx[i, j], h]), then permute
    # to [t, jc, i-half, j, (x0 i-half | x1 i-half)]
    eb = np.exp(tbl)[idx]                                    # [i, j, H] f32
    ebt = eb.transpose(2, 1, 0)                              # [H, Nj, Ni]
    e6 = ebt.reshape(6, 2, NT, 128, 2, 512)                  # [t,x,jc,j,a,i]
    ebt_dev = np.ascontiguousarray(
        e6.transpose(0, 2, 4, 3, 1, 5)).reshape(96, 128, N).astype(BF16)

    shared = {
        "wqkvt": wqkvt_dev,
        "qkvb": qkvb,
        "vb": v_bias.astype(np.float32),
        "wprojt": wprojt,
        "pbias": proj_bias.astype(np.float32),
        "ebt": ebt_dev,
    }
    in_maps = []
    for b in range(B):
        m = dict(shared)
        m["xt"] = np.ascontiguousarray(x[b].T).astype(BF16)  # [C, N]
        in_maps.append(m)
    return in_maps


def kernel(**inputs):
    from concourse.bass_utils import run_bass_kernel_spmd

    nc = _get_nc()
    in_maps = prep_inputs(**inputs)
    res = run_bass_kernel_spmd(nc, in_maps, list(range(N_CORES)),
                               trace=False)
    _cache["last_result"] = res
    out = np.stack([res.results[b]["out"] for b in range(B)], axis=0)
    return out.astype(np.float32)


def kernel_profiled(**inputs):
    """Same as kernel() but with NTFF tracing; returns (out, BassKernelResults)."""
    from concourse.bass_utils import run_bass_kernel_spmd

    nc = _get_nc()
    in_maps = prep_inputs(**inputs)
    res = run_bass_kernel_spmd(nc, in_maps, list(range(N_CORES)), trace=True)
    out = np.stack([res.results[b]["out"] for b in range(B)], axis=0)
    return out.astype(np.float32), res


# revision 34
# speedup vs baseline: 1.2065x; 1.0123x over previous
"""Trainium2 Bass kernel: multi-head attention with relative-position bias.

Problem shapes: x [8, 1024, 768], H=12 heads, d=64.
Strategy: data-parallel over batch (1 element per NeuronCore, 8 cores).
All matmuls in bf16 (f32 PSUM accumulation). Host prep:
  - weights transposed to [C, *] feature-major; q-scale folded into Wq/q_bias
  - qkv weight columns reordered to [q0|k0|q1|k1|...|q5|k5|v] so the q0/k0
    slices (needed first) arrive in a small leading DMA
  - relative-position bias gather done as exp(table)[idx] -> bf16, streamed
    from HBM and folded into softmax multiplicatively:
    softmax(s + b) = norm(exp(s) * exp(b))   (no row-max needed: |s| < ~10)
  - eb tiles permuted to [t, jc, i-half, j, (x0 512 | x1 512)] to match the
    score-psum layout below
Attention computed transposed (sT[j, i]) so softmax sums run along the PE
contraction: the PV matmul uses stationary [v | 1], giving the denominator as
an extra psum row for free.

ALL matmuls run in the single 128x128 PE tiling mode: the score matmuls use
K=128 stationaries zero-padded per head (kz tiles: head x's k dims in rows
x*64..x*64+64, zeros elsewhere; the moving operand is the full two-head q
tile, the zero rows kill the cross-head terms). This avoids the
64x128-row-tiling <-> 128x128 mode switches (each forces a PE array drain)
that otherwise occur 4x per score group.

Phase B is a single global software pipeline over all 48 (head-pair, jc)
groups. Score psum tiles are grouped BY i-HALF: tileA holds
[s_x0(i<512) | s_x1(i<512)]; one exp (scalar engine) and one bf16 multiply
(vs the host-permuted eb tile, on the vector engine) cover both heads.
PV matmuls trail by ~3 groups via a pending queue; v-projection tiles and
the remaining q/k projection tiles are inserted as PE filler inside early
groups. Normalization uses a DRAM bounce to transpose the denominator row
into a [16,128] reciprocal; evacuation copies are split scalar/vector.
"""
import sys
import numpy as np

sys.path.insert(0, "/opt/trn_rl_repo")

import ml_dtypes

BF16 = ml_dtypes.bfloat16

B, N, C = 8, 1024, 768
H, D = 12, 64
N_CORES = 8
NT = N // 128        # 8 token tiles
CT = C // 128        # 6 feature tiles
OT = 3 * C // 128    # 18 qkv output feature tiles

_cache = {}


def _install_axon_shim():
    """The image's antenv lacks axon_hooks; register the NTFF profile hook so
    run_bass_kernel_spmd(trace=True) works. Safe no-op outside axon."""
    import types

    if "antenv.axon_hooks" not in sys.modules:
        try:
            import antenv
            from trn_agent_boot.trn_boot import _ntff_profile_via_ctypes
        except ImportError:
            return
        mod = types.ModuleType("antenv.axon_hooks")
        _hook = [None]
        mod.set_axon_ntff_profile_hook = lambda h: _hook.__setitem__(0, h)
        mod.get_axon_ntff_profile_hook = lambda: _hook[0]
        sys.modules["antenv.axon_hooks"] = mod
        antenv.axon_hooks = mod
        try:
            mod.set_axon_ntff_profile_hook(
                _ntff_profile_via_ctypes("/opt/axon/libaxon_pjrt.so")
            )
        except Exception:
            pass
    from concourse import bass_utils

    bass_utils.upload_artifacts = lambda tmpdir: tmpdir


def build_nc():
    from concourse import bacc, mybir, tile
    from concourse.tile import add_dep_helper

    f32 = mybir.dt.float32
    bf16 = mybir.dt.bfloat16
    AF = mybir.ActivationFunctionType

    nc = bacc.Bacc("TRN2", target_bir_lowering=False, debug=False,
                   num_devices=N_CORES)

    xt_d = nc.dram_tensor("xt", [C, N], bf16, kind="ExternalInput")
    # device column order: [q0|k0|q1|k1|...|q5|k5 | v(768)]
    wqkvt_d = nc.dram_tensor("wqkvt", [C, 3 * C], bf16, kind="ExternalInput")
    qkvb_d = nc.dram_tensor("qkvb", [3 * C], f32, kind="ExternalInput")
    vb_d = nc.dram_tensor("vb", [C], f32, kind="ExternalInput")
    wprojt_d = nc.dram_tensor("wprojt", [C, C], bf16, kind="ExternalInput")
    pbias_d = nc.dram_tensor("pbias", [C], f32, kind="ExternalInput")
    # eb tiles: [(t*8+jc)*2 + i-half, j, (x0 i-half | x1 i-half)]
    ebt_d = nc.dram_tensor("ebt", [96, 128, N], bf16, kind="ExternalInput")
    out_d = nc.dram_tensor("out", [N, C], bf16, kind="ExternalOutput")

    with tile.TileContext(nc) as tc:
        with (
            tc.tile_pool(name="persist", bufs=1) as persist,
            tc.tile_pool(name="work", bufs=1) as work,
            tc.tile_pool(name="dram", bufs=2, space="DRAM") as dpool,
            tc.tile_pool(name="psum", bufs=1, space="PSUM") as psum,
        ):
            # ---- resident tiles ----
            # q feature-major [128=(x,d), N] per pair
            q_sb = [persist.tile([128, N], bf16, tag=f"q{i}", name=f"q{i}")
                    for i in range(6)]
            # k stationaries, zero-padded per head: kz[2t+x] holds head x's
            # k dims in rows x*64..x*64+64, zeros elsewhere -> K=128 matmuls
            kz_sb = [persist.tile([128, N], bf16, tag=f"kz{i}", name=f"kz{i}")
                     for i in range(12)]
            # v token-major, 12 groups of (64 vals + 1 one) per token tile
            v_sb = [persist.tile([128, H * 65], bf16, tag=f"v{i}", name=f"v{i}")
                    for i in range(NT)]
            # attention output (pre-proj), feature-major
            ot_sb = [persist.tile([128, N], bf16, tag=f"ot{i}", name=f"ot{i}")
                     for i in range(CT)]
            # small constants
            qkvb_sb = persist.tile([128, OT], f32, tag="qkvb")
            vb_bc = persist.tile([128, C], f32, tag="vb_bc")
            pb_bc = persist.tile([128, C], f32, tag="pb_bc")
            ones_sb = persist.tile([128, 512], bf16, tag="ones")
            nc.vector.memset(ones_sb[:], 1.0)
            # zero halves of the kz stationaries (written once). GPSIMD:
            # keeps the vector queue free for the ramp-phase evacuations.
            for t in range(6):
                nc.gpsimd.memset(kz_sb[2 * t + 0][64:128, :], 0.0)
                nc.gpsimd.memset(kz_sb[2 * t + 1][0:64, :], 0.0)

            # ---- input DMAs, priority-ordered ----
            # first: x and the narrow q0/k0 weight slice, so compute starts asap
            xt_sb = [work.tile([128, N], bf16, tag=f"xt{i}", name=f"xt{i}")
                     for i in range(CT)]
            wqkv0_sb = [work.tile([128, 256], bf16, tag=f"wq0{i}", name=f"wq0{i}")
                        for i in range(CT)]
            wv_sb = [work.tile([128, C], bf16, tag=f"wv{i}", name=f"wv{i}")
                     for i in range(CT)]
            wqk2_sb = [work.tile([128, 2 * C - 256], bf16, tag=f"wqr{i}",
                                 name=f"wqr{i}") for i in range(CT)]
            first_dmas = []
            for ct in range(CT):
                first_dmas.append(nc.sync.dma_start(
                    xt_sb[ct][:], xt_d.ap()[ct * 128:(ct + 1) * 128, :]))
                first_dmas.append(nc.sync.dma_start(
                    wqkv0_sb[ct][:], wqkvt_d.ap()[ct * 128:(ct + 1) * 128, 0:256]))
            # qkvb needed first (q0 evac ~16us); the wide vb/pb broadcast
            # DMAs (~2us each) are deferred into the loop so they don't delay
            # the first eb tiles
            nc.sync.dma_start(qkvb_sb[:], qkvb_d.ap().rearrange("(t p) -> p t", p=128))

            # wv / wqk2 / wproj DMAs are deferred into the group loop (queue
            # order: x+wqkv0, eb(0), wv, eb(1), eb(2)+wqk2, ..., eb(12)+wproj)
            # so the eb stream starts as early as possible
            wproj_sb = [persist.tile([128, C], bf16, tag=f"wp{i}", name=f"wp{i}")
                        for i in range(CT)]

            # ================= qkv projection emitters =================
            def qk_w_ap(ct, o):
                if o < 2:
                    return wqkv0_sb[ct][:, o * 128:(o + 1) * 128]
                return wqk2_sb[ct][:, (o - 2) * 128:(o - 1) * 128]

            # q,k feature-major: qkvT[o, n] = sum_c WT[c, o] * xT[c, n]
            # emitted in two halves so an insert can straddle a score group
            def emit_qk_half(o, half, ps):
                for ct in (range(0, 3) if half == 0 else range(3, CT)):
                    for h2 in range(2):
                        nc.tensor.matmul(
                            ps[:, h2 * 512:(h2 + 1) * 512],
                            qk_w_ap(ct, o),
                            xt_sb[ct][:, h2 * 512:(h2 + 1) * 512],
                            start=(ct == 0), stop=(ct == CT - 1),
                            skip_group_check=True,
                        )
                if half == 1:
                    t = o // 2
                    if o % 2 == 0:  # q: per-partition bias add
                        nc.vector.tensor_scalar_add(q_sb[t][:], ps[:],
                                                    qkvb_sb[:, o:o + 1])
                    else:  # k: bias is identically zero -> plain copies into
                        # the data rows of the two zero-padded stationaries.
                        # (Tried on scalar: delays the exp stream there and
                        # costs more than it saves on the vector queue.)
                        nc.vector.tensor_copy(kz_sb[2 * t + 0][0:64, :],
                                              ps[0:64, :])
                        nc.vector.tensor_copy(kz_sb[2 * t + 1][64:128, :],
                                              ps[64:128, :])

            def emit_qk_tile(o):
                ps = psum.tile([128, N], f32, tag="big", name="psa", bufs=2)
                emit_qk_half(o, 0, ps)
                emit_qk_half(o, 1, ps)

            # v token-major: v[n, vd] = sum_c xT[c, n] * WT[c, 2C+vd]
            # NOTE: 384-wide matmul outputs must start at 512-aligned psum
            # offsets (a matmul output may not cross a 2KB PSUM bank).

            def emit_v_half(nt, half, ps):
                for ct in (range(0, 3) if half == 0 else range(3, CT)):
                    for g2 in range(2):
                        nc.tensor.matmul(
                            ps[:, g2 * 512:g2 * 512 + 384],
                            xt_sb[ct][:, nt * 128:(nt + 1) * 128],
                            wv_sb[ct][:, g2 * 384:(g2 + 1) * 384],
                            start=(ct == 0), stop=(ct == CT - 1),
                            skip_group_check=True,
                        )
                if half == 1:
                    emit_v_evac(nt, ps)

            def emit_v_tile(nt):
                # v tiles 0..4 use the pv psum banks (idle until the first
                # PV group pops at slot PV_LAG): keeps the "big" ring free
                # for the score tiles during the pair-0 ramp
                if nt <= 4:
                    ps = psum.tile([128, N], f32, tag=f"pv{nt % 2}",
                                   name=f"psv{nt}", bufs=1)
                else:
                    ps = psum.tile([128, N], f32, tag="big", name="psv",
                                   bufs=2)
                emit_v_half(nt, 0, ps)
                emit_v_half(nt, 1, ps)

            def emit_v_evac(nt, ps):
                v_view = v_sb[nt][:].rearrange("p (g c) -> p g c", c=65)
                ps_view = (ps[:].rearrange("p (g c) -> p g c", g=2)[:, :, 0:384]
                           .rearrange("p g (h c) -> p g h c", c=64))
                nc.vector.tensor_add(
                    v_view[:, :, 0:64].rearrange("p (g h) c -> p g h c", g=2),
                    ps_view,
                    vb_bc[:].rearrange("p (g h c) -> p g h c", g=2, c=64),
                )
                nc.vector.memset(v_view[:, :, 64:65], 1.0)

            # PE warm-up: a few junk matmuls bridging the gap until the first
            # x/weight tiles land; the qkv matmuls then keep the PE busy
            # through the HAM warm-up window themselves. More warm-up would
            # head-of-line block the real work (PE queue is strict FIFO).
            warm_ps = psum.tile([128, N], f32, tag="big", name="warm", bufs=2)
            for _ in range(4):
                nc.tensor.matmul(warm_ps[:, 0:512], ones_sb[:, 0:128],
                                 ones_sb[:, 0:512], start=True, stop=True,
                                 skip_group_check=True)

            # prologue: only q0/k0 (needs just the narrow weight slice)
            emit_qk_tile(0)
            emit_qk_tile(1)

            # ================= Phase B: attention (global pipeline) ========
            groups = [(t, jc) for t in range(6) for jc in range(NT)]  # 48
            pms = {}           # (t, jc) -> [pmA, pmB]  (A/B = i-halves)
            pv = {}            # t -> [x] psum tiles ([128,1024], rows 0:65)
            pv_pending = []    # group indices whose PV is not yet emitted
            # Emit PV for group g at slot >= g + PV_LAG. The PE queue is
            # strict FIFO: a PV matmul whose pm isn't ready head-of-line
            # blocks everything behind it. Lag 5 puts the last groups' PVs
            # behind the NEXT pair's score matmuls in the queue, so the PE
            # keeps streaming across pair boundaries while exp/mul catch up.
            # (pm pool: live range is ~6 groups x 2 tiles <= 14 bufs.)
            PV_LAG = 5
            # PE filler inserted inside groups:
            #   v tiles at (0, jc); q/k tiles for pair t+1 inside pair t
            inserts = {}
            for jc in range(NT):
                inserts.setdefault((0, jc), []).append(("v", jc))
            inserts.setdefault((0, 5), []).append(("qk", 2))
            inserts.setdefault((0, 6), []).append(("qk", 3))
            for t in range(1, 5):
                inserts.setdefault((t, 2), []).append(("qk", 2 * (t + 1)))
                # (t, 5) leaves the k evac a full group of margin before the
                # next pair's first score matmul reads the kz tiles
                inserts.setdefault((t, 5), []).append(("qk", 2 * (t + 1) + 1))

            # Deferred norm multiplies: (emit_at_slot, t, u2, rb2). The norm
            # muls write ot_sb[t], which is not read until phase C — emitting
            # them 4 slots late keeps them out of the DVE FIFO ahead of the
            # pair-end eb-multiplies that feed the lagged PV matmuls.
            norm_muls_q = []

            def emit_norm_muls(t, u2, rb2):
                for x in range(2):
                    nc.vector.tensor_mul(
                        ot_sb[t][x * 64:(x + 1) * 64, :],
                        u2[0:64, x * 1024:(x + 1) * 1024],
                        rb2[:, x * 1024:(x + 1) * 1024],
                    )

            def emit_pv_group(gi, slot):
                t, jc = groups[gi]
                if jc == 0:
                    pv[t] = [psum.tile([128, N], f32, tag=f"pv{x}",
                                       name=f"pv{x}", bufs=1)
                             for x in range(2)]
                for x in range(2):
                    g = 2 * t + x
                    for ic in range(2):
                        nc.tensor.matmul(
                            pv[t][x][0:65, ic * 512:(ic + 1) * 512],
                            v_sb[jc][:, g * 65:(g + 1) * 65],
                            pms[(t, jc)][ic][:, x * 512:(x + 1) * 512],
                            start=(jc == 0), stop=(jc == NT - 1),
                            skip_group_check=True,
                        )
                if jc == NT - 1:
                    norm_muls_q.append((slot + 4,) + emit_norm(t))

            def emit_norm(t):
                # evacuate both pv tiles into one wide staging tile (cols
                # x-major); row 64 holds the denominators -> one DMA to DRAM,
                # one [16,128] reshape, one reciprocal, one wide stride-0
                # broadcast back, two scaling multiplies.
                u2 = work.tile([65, 2048], bf16, tag="ustage", name="ustage",
                               bufs=2)
                nc.scalar.copy(u2[0:65, 0:1024], pv[t][0][0:65, :])
                nc.vector.tensor_copy(u2[0:65, 1024:2048], pv[t][1][0:65, :])
                # denominator row -> DRAM bounce -> [16,128] compact ->
                # reciprocal -> DRAM bounce -> broadcast across 64 partitions
                # (partition-redistributing / stride-0 APs require DRAM)
                denom_d = dpool.tile([1, 2048], bf16, tag="denom_d",
                                     name="denom_d")
                nc.sync.dma_start(denom_d[0:1, :], u2[64:65, :])
                dstage = work.tile([16, 128], bf16, tag="dstage",
                                   name="dstage", bufs=2)
                nc.sync.dma_start(
                    dstage[:],
                    denom_d[:].rearrange("a b -> (a b)").rearrange(
                        "(p c) -> p c", p=16))
                rstage = work.tile([16, 128], bf16, tag="rstage",
                                   name="rstage", bufs=2)
                with nc.allow_low_precision("softmax denom recip, 2e-2 gate"):
                    nc.vector.reciprocal(rstage[:], dstage[:])
                rd = dpool.tile([16, 128], bf16, tag="rd", name="rd")
                nc.sync.dma_start(rd[:], rstage[:])
                rb2 = work.tile([64, 2048], bf16, tag="rb", name="rb", bufs=2)
                nc.sync.dma_start(
                    rb2[:],
                    rd[:].rearrange("p c -> (p c)").unsqueeze(0)
                    .to_broadcast([64, 2048]))
                return (t, u2, rb2)  # muls deferred (see norm_muls_q)

            for gi, (t, jc) in enumerate(groups):
                while norm_muls_q and norm_muls_q[0][0] <= gi:
                    _, nt_, nu2, nrb = norm_muls_q.pop(0)
                    emit_norm_muls(nt_, nu2, nrb)
                # eb bias tiles for this group (one per i-half, both heads)
                eb = [work.tile([128, N], bf16, tag="eb", name="eb", bufs=12)
                      for _ in range(2)]
                if gi == 0:
                    # wv before eb(0): the v0 insert consumes wv at ~11us,
                    # while the first eb tile isn't read until ~16us
                    for ct in range(CT):
                        nc.sync.dma_start(
                            wv_sb[ct][:],
                            wqkvt_d.ap()[ct * 128:(ct + 1) * 128, 2 * C:])
                    nc.sync.dma_start(
                        vb_bc[:], vb_d.ap().unsqueeze(0).to_broadcast([128, C]))
                for a in range(2):
                    nc.sync.dma_start(
                        eb[a][:], ebt_d.ap()[(t * NT + jc) * 2 + a, :, :])
                if gi == 2:
                    # remaining q/k weights: first consumer is the ("qk", 2)
                    # insert at group 5; enqueue behind the first few eb tiles
                    for ct in range(CT):
                        nc.sync.dma_start(
                            wqk2_sb[ct][:],
                            wqkvt_d.ap()[ct * 128:(ct + 1) * 128, 256:2 * C])
                if gi == 12:
                    # proj weights + bias: needed only in phase C; enqueue
                    # behind the first dozen eb tiles
                    for ct in range(CT):
                        nc.sync.dma_start(
                            wproj_sb[ct][:],
                            wprojt_d.ap()[ct * 128:(ct + 1) * 128, :])
                    nc.sync.dma_start(
                        pb_bc[:],
                        pbias_d.ap().unsqueeze(0).to_broadcast([128, C]))
                # scores grouped by i-half: tile a holds both heads' scores
                # for i in [a*512, (a+1)*512). K=128 matmuls (zero-padded
                # stationary) keep the PE in plain 128x128 mode; PE filler
                # (v / later q,k projection tiles) is emitted between the two
                # score tiles so it streams while exp(A) runs.
                qs = []
                for a in range(2):
                    q = psum.tile([128, N], f32, tag="big", name=f"qs{a}",
                                  bufs=2)
                    qs.append(q)
                    for x in range(2):
                        nc.tensor.matmul(
                            q[:, x * 512:(x + 1) * 512],
                            kz_sb[2 * t + x][:, jc * 128:(jc + 1) * 128],
                            q_sb[t][:, a * 512:(a + 1) * 512],
                            start=True, stop=True,
                            skip_group_check=True,
                        )
                    if a == 0:
                        for kind, idx in inserts.get((t, jc), []):
                            if kind == "v":
                                emit_v_tile(idx)
                            else:
                                emit_qk_tile(idx)
                pml = []
                for a in range(2):
                    pe = work.tile([128, N], bf16, tag="pe", name="pe", bufs=6)
                    nc.scalar.activation(pe[:], qs[a][:], AF.Exp)
                    pm = work.tile([128, N], bf16, tag="pm", name="pm", bufs=14)
                    # NOTE: GPSIMD offload of these multiplies was tried and
                    # is a net loss — GPSIMD shares the SBUF port with the
                    # DVE, and concurrent DVE tensor_tensor ops slowed 3.5x.
                    nc.vector.tensor_mul(pm[:], pe[:], eb[a][:])
                    pml.append(pm)
                pms[(t, jc)] = pml
                pv_pending.append(gi)
                # lagged PV emission (<=2 groups per slot keeps PE smooth)
                emitted = 0
                while pv_pending and pv_pending[0] <= gi - PV_LAG and emitted < 2:
                    emit_pv_group(pv_pending.pop(0), gi)
                    emitted += 1
            # ================= Phase C: output projection =================
            # Interleave the final PV drain with ct=0..4 accumulation for the
            # first token tiles; nt=2/3 reuse the pv psum banks freed by the
            # last pair's norm copies. ct=5 (gated on ot_sb[5]) finishes each
            # held tile afterwards; nt=4..7 then run all six ct in one pass.
            def emit_proj_mms(nt, ps, cts, first, last):
                for ct in cts:
                    for oc in range(2):
                        nc.tensor.matmul(
                            ps[:, oc * 512:oc * 512 + 384],
                            ot_sb[ct][:, nt * 128:(nt + 1) * 128],
                            wproj_sb[ct][:, oc * 384:(oc + 1) * 384],
                            start=(ct == first), stop=(ct == last),
                            skip_group_check=True,
                        )

            def emit_proj_out(nt, ps):
                osb = work.tile([128, C], bf16, tag="osb", name="osb", bufs=3)
                ps_view = ps[:].rearrange("p (g c) -> p g c", g=2)[:, :, 0:384]
                nc.vector.tensor_add(
                    osb[:].rearrange("p (g c) -> p g c", g=2), ps_view,
                    pb_bc[:].rearrange("p (g c) -> p g c", g=2))
                nc.sync.dma_start(out_d.ap()[nt * 128:(nt + 1) * 128, :], osb[:])

            def proj_ps(nt):
                tag = ["big", "big", "pv0", "pv1"][nt % 4]
                return psum.tile([128, N], f32, tag=tag, name=f"psc{nt}",
                                 bufs=(2 if tag == "big" else 1))

            held4 = []
            # flush norm muls still pending from the main loop (pair 4)
            while norm_muls_q:
                _, nt_, nu2, nrb = norm_muls_q.pop(0)
                emit_norm_muls(nt_, nu2, nrb)
            while pv_pending:
                emit_pv_group(pv_pending.pop(0), len(groups) + 8)
                if len(held4) < 2:  # nt 0/1 on the "big" ring during drain
                    nt = len(held4)
                    ps = proj_ps(nt)
                    emit_proj_mms(nt, ps, range(5), 0, CT - 1)
                    held4.append((nt, ps))
            # pair 5's norm muls (queued during the drain) go out now —
            # before the ct=5 projections that read ot_sb[5]
            while norm_muls_q:
                _, nt_, nu2, nrb = norm_muls_q.pop(0)
                emit_norm_muls(nt_, nu2, nrb)
            for nt in (2, 3):  # pv banks free once norm(5)'s copies ran
                ps = proj_ps(nt)
                emit_proj_mms(nt, ps, range(5), 0, CT - 1)
                held4.append((nt, ps))
            for nt, ps in held4:
                emit_proj_mms(nt, ps, [5], 0, CT - 1)
                emit_proj_out(nt, ps)
            for nt in range(4, NT):
                ps = proj_ps(nt)
                emit_proj_mms(nt, ps, range(CT), 0, CT - 1)
                emit_proj_out(nt, ps)

    nc.compile()
    return nc


def _get_nc():
    if "nc" not in _cache:
        _install_axon_shim()
        _cache["nc"] = build_nc()
    return _cache["nc"]


def prep_inputs(x, relative_position_index, qkv_weight, q_bias, v_bias,
                proj_weight, proj_bias, rel_pos_bias_table):
    """Host-side layout prep shared by all cores + per-core shards."""
    x = np.asarray(x, np.float32)
    idx = np.asarray(relative_position_index)
    qkv_weight = np.asarray(qkv_weight, np.float32)
    q_bias = np.asarray(q_bias, np.float32)
    v_bias = np.asarray(v_bias, np.float32)
    proj_weight = np.asarray(proj_weight, np.float32)
    proj_bias = np.asarray(proj_bias, np.float32)
    tbl = np.asarray(rel_pos_bias_table, np.float32)

    scale = (C // H) ** (-0.5)
    wq = qkv_weight.copy()
    wq[:C, :] *= scale  # fold softmax scale into q projection
    wqkvt = np.ascontiguousarray(wq.T)  # [C, 3C] cols: q(768) k(768) v(768)
    # device column order: [q0|k0|q1|k1|...|q5|k5|v]
    cols = []
    for t in range(6):
        cols.append(wqkvt[:, t * 128:(t + 1) * 128])
        cols.append(wqkvt[:, C + t * 128:C + (t + 1) * 128])
    cols.append(wqkvt[:, 2 * C:])
    wqkvt_dev = np.ascontiguousarray(np.concatenate(cols, axis=1)).astype(BF16)

    qb_s = q_bias * scale
    qkvb_parts = []
    for t in range(6):
        qkvb_parts.append(qb_s[t * 128:(t + 1) * 128])
        qkvb_parts.append(np.zeros(128, np.float32))
    qkvb_parts.append(v_bias)
    qkvb = np.concatenate(qkvb_parts).astype(np.float32)

    wprojt = np.ascontiguousarray(proj_weight.T).astype(BF16)  # [C, C]

    # exp(bias) gather: ebt[h, j, i] = exp(table[idx[i, j], h]), then permute
    # to [t, jc, i-half, j, (x0 i-half | x1 i-half)]
    eb = np.exp(tbl)[idx]                                    # [i, j, H] f32
    ebt = eb.transpose(2, 1, 0)                              # [H, Nj, Ni]
    e6 = ebt.reshape(6, 2, NT, 128, 2, 512)                  # [t,x,jc,j,a,i]
    ebt_dev = np.ascontiguousarray(
        e6.transpose(0, 2, 4, 3, 1, 5)).reshape(96, 128, N).astype(BF16)

    shared = {
        "wqkvt": wqkvt_dev,
        "qkvb": qkvb,
        "vb": v_bias.astype(np.float32),
        "wprojt": wprojt,
        "pbias": proj_bias.astype(np.float32),
        "ebt": ebt_dev,
    }
    in_maps = []
    for b in range(B):
        m = dict(shared)
        m["xt"] = np.ascontiguousarray(x[b].T).astype(BF16)  # [C, N]
        in_maps.append(m)
    return in_maps


def kernel(**inputs):
    from concourse.bass_utils import run_bass_kernel_spmd

    nc = _get_nc()
    in_maps = prep_inputs(**inputs)
    res = run_bass_kernel_spmd(nc, in_maps, list(range(N_CORES)),
                               trace=False)
    _cache["last_result"] = res
    out = np.stack([res.results[b]["out"] for b in range(B)], axis=0)
    return out.astype(np.float32)


def kernel_profiled(**inputs):
    """Same as kernel() but with NTFF tracing; returns (out, BassKernelResults)."""
    from concourse.bass_utils import run_bass_kernel_spmd

    nc = _get_nc()
    in_maps = prep_inputs(**inputs)
    res = run_bass_kernel_spmd(nc, in_maps, list(range(N_CORES)), trace=True)
    out = np.stack([res.results[b]["out"] for b in range(B)], axis=0)
    return out.astype(np.float32), res
